# revision 9
# baseline (speedup 1.0000x reference)
"""Trainium2 Bass kernel for nn_DiagOpLayer (CG solve on masked tridiagonal op).

Math: reference runs MAX_ITER=20 CG iterations on K = G^T (D^T W^2 D) G where
G scatters the n_miss unknowns into the full n-length signal and D is the
first-difference operator.  We run the CG in the FULL space on masked vectors:
all CG iterates stay exactly supported on the unknown-mask, so
  K_full(p) = unk * D^T(w^2 * D p)
reproduces the compressed iteration exactly (off-mask entries stay 0.0).

Layout per batch row: [128 partitions, W] f32 where partition p holds global
positions [2048*p - H, 2048*p + 2047 + H] (H-halo each side, redundantly).
All ops are elementwise or +-1 shifts along the free dim, so halo validity
shrinks by <=1 column per side per CG iteration and no cross-partition traffic
is ever needed inside the loop (H=24 > 20 iterations + pre/post shifts).

Sharding: data-parallel, 2 batch rows per core across 8 cores; unknown-mask
replicated.  Dot products reduce per-row on-device (ACT square-accumulate ->
PE ones-matmul); alpha/beta scalar chains run on [1,1] tiles; broadcasts to
[128,1] via K=1 PE matmuls into PSUM.
"""

import sys

import numpy as np

for _p in ("/opt/trn_rl_repo",):
    if _p not in sys.path:
        sys.path.insert(0, _p)

# Problem constants (hardcoded per spec; kernel.py must be self-contained).
B = 16
N = 262144
M = N - 1
NMISS = 131072
MAX_ITER = 20
EPS_W = 1e-3
CLAMP_MIN = 1e-4
CLAMP_MAX = 1e4
NCORES = 8
ROWS = B // NCORES  # 2 batch rows per core
C = 2048            # core columns per partition: 128 * 2048 = N
H = 24              # halo columns each side (> 20 iters + pre/post shifts)
W = C + 2 * H
PADN = N + 2 * H    # padded HBM row length so the halo'd DMA AP stays in-bounds
PAD_U = -60.0       # softplus(-60) ~ 9e-27 -> padded w = EPS_W; contributes
                    # O(w^2)=1e-6-weighted terms only at out-of-range edges,
                    # which are multiplied by exact-0 mask/pad values or are
                    # ~1e-12 relative -- far below fp32 noise.

_CACHE = {}


def _build_program(debug=False, enable_asserts=False, repeat=1):
    """Build the SPMD Bass/Tile program for one core (2 batch rows).

    repeat>1 wraps the whole body in a hardware For_i loop (timing only:
    difference two repeat values to cancel RPC/launch overhead).
    """
    from contextlib import ExitStack

    import concourse.bass as bass
    import concourse.tile as tile
    from concourse import bacc, mybir

    f32 = mybir.dt.float32
    Alu = mybir.AluOpType
    Act = mybir.ActivationFunctionType

    nc = bacc.Bacc(
        "TRN2",
        target_bir_lowering=False,
        debug=debug,
        enable_asserts=enable_asserts,
    )

    u_in = nc.dram_tensor("u_in", [ROWS, PADN], f32, kind="ExternalInput")
    x_in = nc.dram_tensor("x_in", [ROWS, PADN], f32, kind="ExternalInput")
    unk_in = nc.dram_tensor("unk_in", [PADN], f32, kind="ExternalInput")

    v_out = nc.dram_tensor("v_out", [ROWS, N], f32, kind="ExternalOutput")
    r_out = nc.dram_tensor("r_out", [ROWS, N], f32, kind="ExternalOutput")
    w_out = nc.dram_tensor("w_out", [ROWS, N], f32, kind="ExternalOutput")
    phi_out = nc.dram_tensor("phi_out", [ROWS, 1], f32, kind="ExternalOutput")

    def dram_row_ap(handle, row, width):
        return bass.AP(handle, row * (PADN if width == W else N), [[C, 128], [1, width]])

    with tile.TileContext(nc) as tc, ExitStack() as ctx:
        big = ctx.enter_context(tc.tile_pool(name="big", bufs=1))
        tiny = ctx.enter_context(tc.tile_pool(name="tiny", bufs=1))
        parts = ctx.enter_context(tc.tile_pool(name="parts", bufs=4))
        # PSUM has 8 banks; each tile takes a full bank -> 4 bufs per tag x 2 tags.
        psum = ctx.enter_context(
            tc.tile_pool(name="psum", bufs=4, space=bass.MemorySpace.PSUM)
        )

        unk_t = big.tile([128, W], f32, tag="unk")
        nc.sync.dma_start(out=unk_t[:], in_=bass.AP(unk_in, 0, [[C, 128], [1, W]]))
        ones_row = tiny.tile([1, 128], f32, tag="ones_row")
        nc.vector.memset(ones_row[:], 1.0)
        ones_col = tiny.tile([128, 1], f32, tag="ones_col")
        nc.vector.memset(ones_col[:], 1.0)

        CORE = slice(H, H + C)

        class Row:
            pass

        rows = []
        for r in range(ROWS):
            R = Row()
            rows.append(R)
            R.u = big.tile([128, W], f32, tag=f"u{r}")     # u logits -> Kp scratch
            R.x = big.tile([128, W], f32, tag=f"x{r}")     # x -> xk -> ACT junk
            R.w = big.tile([128, W], f32, tag=f"w{r}")
            R.b = big.tile([128, W], f32, tag=f"b{r}")
            R.V = big.tile([128, W], f32, tag=f"V{r}")
            R.rc = big.tile([128, W], f32, tag=f"rc{r}")
            R.p = big.tile([128, W], f32, tag=f"p{r}")
            R.s1 = big.tile([128, W], f32, tag=f"s1_{r}")
            R.s2 = big.tile([128, W], f32, tag=f"s2_{r}")
            R.rs_a = tiny.tile([1, 1], f32, tag=f"rs_a{r}")
            R.rs_b = tiny.tile([1, 1], f32, tag=f"rs_b{r}")
            R.den = tiny.tile([1, 1], f32, tag=f"den{r}")
            R.alpha = tiny.tile([1, 1], f32, tag=f"alpha{r}")
            R.nalpha = tiny.tile([1, 1], f32, tag=f"nalpha{r}")
            R.beta = tiny.tile([1, 1], f32, tag=f"beta{r}")
            R.phi = tiny.tile([1, 1], f32, tag=f"phi{r}")

        from contextlib import nullcontext

        loop_cm = tc.For_i(0, repeat, 1) if repeat > 1 else nullcontext()
        loop_cm.__enter__()

        # ---------- preamble ----------
        for r in range(ROWS):
            R = rows[r]
            nc.sync.dma_start(out=R.u[:], in_=dram_row_ap(u_in, r, W))
            nc.sync.dma_start(out=R.x[:], in_=dram_row_ap(x_in, r, W))
            nc.gpsimd.memset(R.V[:], 0.0)
            nc.gpsimd.memset(R.p[:], 0.0)
            nc.gpsimd.memset(R.rc[:], 0.0)

            # w = clip(softplus(u) + EPS_W, CLAMP_MIN, CLAMP_MAX)
            # No Ln/Softplus ACT table on gen3 -> Newton on exp(s) = 1 + e^u:
            # s0 = relu(u); s += (1+e^u)*e^-s - 1 (quadratic, 4 iters to fp32).
            # All ACT funcs used (Exp/Relu/Square/Copy) share one table.
            nc.vector.tensor_scalar_min(out=R.u[:], in0=R.u[:], scalar1=80.0)
            nc.scalar.activation(out=R.s2[:], in_=R.u[:], func=Act.Exp)
            nc.vector.tensor_scalar_add(out=R.s2[:], in0=R.s2[:], scalar1=1.0)
            nc.scalar.activation(out=R.w[:], in_=R.u[:], func=Act.Relu)
            for _ in range(4):
                nc.scalar.activation(out=R.s1[:], in_=R.w[:], func=Act.Exp, scale=-1.0)
                nc.vector.tensor_mul(out=R.s1[:], in0=R.s2[:], in1=R.s1[:])
                nc.vector.scalar_tensor_tensor(
                    out=R.w[:], in0=R.s1[:], scalar=-1.0, in1=R.w[:],
                    op0=Alu.add, op1=Alu.add,
                )
            nc.vector.tensor_scalar(
                out=R.w[:], in0=R.w[:], scalar1=EPS_W, scalar2=CLAMP_MIN,
                op0=Alu.add, op1=Alu.max,
            )
            nc.vector.tensor_scalar_min(out=R.w[:], in0=R.w[:], scalar1=CLAMP_MAX)

            # xk = x * (1 - unk) ;  b = -D(xk):  b[i] = xk[i] - xk[i+1]
            nc.gpsimd.tensor_mul(out=R.s1[:], in0=R.x[:], in1=unk_t[:])
            nc.vector.tensor_sub(out=R.x[:], in0=R.x[:], in1=R.s1[:])
            nc.vector.tensor_sub(
                out=R.b[:, 0:W - 1], in0=R.x[:, 0:W - 1], in1=R.x[:, 1:W]
            )
            # rhs = unk * Dt(w2 * b):  tb2 = w*(w*b);  z0[j] = tb2[j-1]-tb2[j]
            nc.vector.tensor_mul(out=R.s2[:, 0:W - 1], in0=R.w[:, 0:W - 1], in1=R.b[:, 0:W - 1])
            nc.vector.tensor_mul(out=R.s1[:, 0:W - 1], in0=R.w[:, 0:W - 1], in1=R.s2[:, 0:W - 1])
            nc.vector.tensor_sub(
                out=R.s2[:, 1:W - 1], in0=R.s1[:, 0:W - 2], in1=R.s1[:, 1:W - 1]
            )
            nc.gpsimd.tensor_mul(
                out=R.rc[:, 1:W - 1], in0=unk_t[:, 1:W - 1], in1=R.s2[:, 1:W - 1]
            )
            nc.scalar.copy(out=R.p[:, 1:W - 1], in_=R.rc[:, 1:W - 1])

            # rs0 = sum(rc^2) over core region
            part = parts.tile([128, 1], f32, tag="part")
            nc.scalar.activation(
                out=R.x[:, 0:C], in_=R.rc[:, CORE], func=Act.Square, accum_out=part[:]
            )
            ps = psum.tile([1, 1], f32, tag="ps_s")
            nc.tensor.matmul(ps[:], part[:], ones_col[:], start=True, stop=True)
            nc.vector.tensor_copy(out=R.rs_a[:], in_=ps[:])

        # ---------- CG loop ----------
        for k in range(MAX_ITER):
            for r in range(ROWS):
                R = rows[r]
                rs_cur = R.rs_a if k % 2 == 0 else R.rs_b
                rs_new = R.rs_b if k % 2 == 0 else R.rs_a

                # tm = D p ; that = w * tm ; pKp = sum(that^2) over core
                nc.vector.tensor_sub(
                    out=R.s1[:, 0:W - 1], in0=R.p[:, 1:W], in1=R.p[:, 0:W - 1]
                )
                nc.vector.tensor_mul(
                    out=R.s2[:, 0:W - 1], in0=R.w[:, 0:W - 1], in1=R.s1[:, 0:W - 1]
                )
                part = parts.tile([128, 1], f32, tag="part")
                nc.scalar.activation(
                    out=R.x[:, 0:C], in_=R.s2[:, CORE], func=Act.Square,
                    accum_out=part[:],
                )
                pkp_ps = psum.tile([1, 1], f32, tag="ps_s")
                nc.tensor.matmul(pkp_ps[:], part[:], ones_col[:], start=True, stop=True)

                # t = w * that ; z[j] = t[j-1] - t[j] ; Kp = unk * z
                nc.vector.tensor_mul(
                    out=R.s1[:, 0:W - 1], in0=R.w[:, 0:W - 1], in1=R.s2[:, 0:W - 1]
                )
                nc.vector.tensor_sub(
                    out=R.s2[:, 1:W - 1], in0=R.s1[:, 0:W - 2], in1=R.s1[:, 1:W - 1]
                )
                nc.gpsimd.tensor_mul(
                    out=R.u[:, 1:W - 1], in0=unk_t[:, 1:W - 1], in1=R.s2[:, 1:W - 1]
                )

                # alpha = rs / (pKp + 1e-30); broadcast alpha and -alpha
                nc.vector.tensor_scalar_add(out=R.den[:], in0=pkp_ps[:], scalar1=1e-30)
                nc.vector.reciprocal(out=R.den[:], in_=R.den[:])
                nc.vector.tensor_mul(out=R.alpha[:], in0=rs_cur[:], in1=R.den[:])
                nc.vector.tensor_scalar_mul(out=R.nalpha[:], in0=R.alpha[:], scalar1=-1.0)
                abc = psum.tile([128, 1], f32, tag="ps_b")
                nc.tensor.matmul(abc[:], ones_row[:], R.alpha[:], start=True, stop=True)
                nabc = psum.tile([128, 1], f32, tag="ps_b")
                nc.tensor.matmul(nabc[:], ones_row[:], R.nalpha[:], start=True, stop=True)

                # V += alpha*p ; rc -= alpha*Kp
                nc.vector.scalar_tensor_tensor(
                    out=R.V[:, 1:W - 1], in0=R.p[:, 1:W - 1], scalar=abc[:],
                    in1=R.V[:, 1:W - 1], op0=Alu.mult, op1=Alu.add,
                )
                nc.vector.scalar_tensor_tensor(
                    out=R.rc[:, 1:W - 1], in0=R.u[:, 1:W - 1], scalar=nabc[:],
                    in1=R.rc[:, 1:W - 1], op0=Alu.mult, op1=Alu.add,
                )

                # rs_new = sum(rc^2); beta = rs_new/(rs+1e-30); p = rc + beta*p
                part2 = parts.tile([128, 1], f32, tag="part")
                nc.scalar.activation(
                    out=R.x[:, 0:C], in_=R.rc[:, CORE], func=Act.Square,
                    accum_out=part2[:],
                )
                rs_ps = psum.tile([1, 1], f32, tag="ps_s")
                nc.tensor.matmul(rs_ps[:], part2[:], ones_col[:], start=True, stop=True)
                nc.vector.tensor_copy(out=rs_new[:], in_=rs_ps[:])
                nc.vector.tensor_scalar_add(out=R.den[:], in0=rs_cur[:], scalar1=1e-30)
                nc.vector.reciprocal(out=R.den[:], in_=R.den[:])
                nc.vector.tensor_mul(out=R.beta[:], in0=rs_new[:], in1=R.den[:])
                bbc = psum.tile([128, 1], f32, tag="ps_b")
                nc.tensor.matmul(bbc[:], ones_row[:], R.beta[:], start=True, stop=True)
                nc.vector.scalar_tensor_tensor(
                    out=R.p[:, 1:W - 1], in0=R.p[:, 1:W - 1], scalar=bbc[:],
                    in1=R.rc[:, 1:W - 1], op0=Alu.mult, op1=Alu.add,
                )

        # ---------- postamble ----------
        for r in range(ROWS):
            R = rows[r]
            # r = D V - b ; phi = sum((w*r)^2) over core ; outputs
            nc.vector.tensor_sub(
                out=R.s1[:, 0:W - 1], in0=R.V[:, 1:W], in1=R.V[:, 0:W - 1]
            )
            nc.vector.tensor_sub(
                out=R.s2[:, 0:W - 1], in0=R.s1[:, 0:W - 1], in1=R.b[:, 0:W - 1]
            )
            nc.vector.tensor_mul(
                out=R.s1[:, 0:W - 1], in0=R.w[:, 0:W - 1], in1=R.s2[:, 0:W - 1]
            )
            part = parts.tile([128, 1], f32, tag="part")
            nc.scalar.activation(
                out=R.x[:, 0:C], in_=R.s1[:, CORE], func=Act.Square, accum_out=part[:]
            )
            phi_ps = psum.tile([1, 1], f32, tag="ps_s")
            nc.tensor.matmul(phi_ps[:], part[:], ones_col[:], start=True, stop=True)
            nc.vector.tensor_copy(out=R.phi[:], in_=phi_ps[:])

            nc.sync.dma_start(out=dram_row_ap(v_out, r, C), in_=R.V[:, CORE])
            nc.sync.dma_start(out=dram_row_ap(r_out, r, C), in_=R.s2[:, CORE])
            nc.sync.dma_start(out=dram_row_ap(w_out, r, C), in_=R.w[:, CORE])
            nc.sync.dma_start(
                out=bass.AP(phi_out, r, [[1, 1], [1, 1]]), in_=R.phi[:]
            )

        loop_cm.__exit__(None, None, None)

    nc.compile()
    return nc


def get_program(debug=False, enable_asserts=False, repeat=1):
    key = (debug, enable_asserts, repeat)
    if key not in _CACHE:
        _CACHE[key] = _build_program(
            debug=debug, enable_asserts=enable_asserts, repeat=repeat
        )
    return _CACHE[key]


def make_in_maps(u_logits, x_full, unknown_idx):
    u = np.ascontiguousarray(np.asarray(u_logits, dtype=np.float32))
    x = np.ascontiguousarray(np.asarray(x_full, dtype=np.float32))
    idx = np.asarray(unknown_idx).astype(np.int64)

    unk = np.zeros(PADN, np.float32)
    unk[H + idx] = 1.0
    u_pad = np.full((B, PADN), PAD_U, np.float32)
    u_pad[:, H:H + M] = u
    x_pad = np.zeros((B, PADN), np.float32)
    x_pad[:, H:H + N] = x

    in_maps = []
    for c in range(NCORES):
        in_maps.append({
            "u_in": u_pad[ROWS * c: ROWS * (c + 1)],
            "x_in": x_pad[ROWS * c: ROWS * (c + 1)],
            "unk_in": unk,
        })
    return in_maps, idx


def assemble_outputs(results, idx):
    phi = np.concatenate([res["phi_out"].reshape(ROWS) for res in results])
    V = np.concatenate([res["v_out"] for res in results], axis=0)
    r = np.concatenate([res["r_out"] for res in results], axis=0)[:, :M]
    w = np.concatenate([res["w_out"] for res in results], axis=0)[:, :M]
    v = V[:, idx]
    return (
        np.ascontiguousarray(phi.astype(np.float32)),
        np.ascontiguousarray(v.astype(np.float32)),
        np.ascontiguousarray(r.astype(np.float32)),
        np.ascontiguousarray(w.astype(np.float32)),
    )


def kernel(u_logits, x_full, unknown_idx):
    from concourse.bass_utils import run_bass_kernel_spmd

    nc = get_program()
    in_maps, idx = make_in_maps(u_logits, x_full, unknown_idx)
    res = run_bass_kernel_spmd(nc, in_maps, list(range(NCORES)))
    return assemble_outputs(res.results, idx)


if __name__ == "__main__":
    rng = np.random.default_rng(0)
    u = rng.standard_normal((B, M), np.float32)
    x = rng.standard_normal((B, N), np.float32)
    idx = np.sort(rng.permutation(N)[:NMISS])
    outs = kernel(u_logits=u, x_full=x, unknown_idx=idx)
    for name, o in zip(["phi", "v", "r", "w"], outs):
        print(name, o.shape, o.dtype, float(np.abs(o).max()))
